# revision 6
# baseline (speedup 1.0000x reference)
"""Causal multihead self-attention with RoPE on 8 TRN2 NeuronCores.

Problem: B=2, S=2048, D=1024, H=16 heads, d_k=64, causal, RoPE theta=10000.

Sharding (Megatron-style, per hint): core c = 4*b + g handles batch b and the
4 heads [4g, 4g+4): Wq/Wk/Wv column-parallel (sliced rows of W since torch
computes x @ W.T), Wo row-parallel; each core emits a partial [S, D] output
and the host sums the 4 partials per batch.

Device kernel (per core), all matmuls in float32r (full-rate fp32 mode):
  A) qT/kT = W' @ x.T in a d-on-partition layout ([128, 2 chunks, S]); the
     d_k dims are permuted (folded into the weights on the host) so chunk 0
     holds the even RoPE lanes (x1) and chunk 1 the odd lanes (x2) for all 4
     heads; RoPE is then 6 full-width DVE ops per (tensor, qtile) against
     host-built cos/sin tables. v is computed in natural [S, d] layout and
     packed as per-head [v | ones] (ones columns produce the softmax sums for
     free during the attention*V matmul).
  B) Per (qtile, head): scoresT tiles [128 kpos, 512 q] via PE (row-group
     packed, 2 heads concurrently), exp on ACT (no max subtraction needed:
     |scores| <= ~40), causal zeroing of diagonal tiles on GPSIMD
     (affine_select), then AV accumulation in PSUM. Softmax normalization:
     1/sums = exp(-ln(sums)) on ACT, lane-shift via SBUF DMA, one DVE mul
     into the yT activation layout.
  C) partial = yT.T @ Wo_slice.T via PE, staged to SBUF, DMA to DRAM.
"""
import sys

sys.path.insert(0, "/opt/trn_rl_repo")

import numpy as np

import concourse.bacc as bacc
import concourse.tile as tile
from concourse import mybir
from concourse.bass_utils import run_bass_kernel_spmd

F32 = mybir.dt.float32
F32R = mybir.dt.float32r
EXP = mybir.ActivationFunctionType.Exp
LN = mybir.ActivationFunctionType.Ln
MUL = mybir.AluOpType.mult
ADD = mybir.AluOpType.add
SUB = mybir.AluOpType.subtract

B, S, D = 2, 2048, 1024
H, DK = 16, 64          # global heads, head dim
HL = 4                  # heads per core
GD = HL * DK            # local width 256
T = S // 512            # 4 q-tiles of 512
C = S // 128            # 16 kpos chunks of 128
DCH = D // 128          # 8 contraction chunks
THETA = 10000.0

_cache = {}


def _build_kernel():
    nc = bacc.Bacc("TRN2", target_bir_lowering=False, debug=False, num_devices=8)

    xT = nc.declare_dram_parameter("xT", [D, S], F32R, isOutput=False)
    wq = nc.declare_dram_parameter("wq", [D, GD], F32R, isOutput=False)
    wk = nc.declare_dram_parameter("wk", [D, GD], F32R, isOutput=False)
    wv = nc.declare_dram_parameter("wv", [D, GD], F32R, isOutput=False)
    wo = nc.declare_dram_parameter("wo", [GD, D], F32R, isOutput=False)
    ccd = nc.declare_dram_parameter("cc", [128, S], F32, isOutput=False)
    ssd = nc.declare_dram_parameter("ss", [128, S], F32, isOutput=False)
    out = nc.declare_dram_parameter("out", [S, D], F32, isOutput=True)

    with tile.TileContext(nc) as tc:
        with (
            tc.tile_pool(name="consts", bufs=1) as consts,
            tc.tile_pool(name="persist", bufs=1) as persist,
            tc.tile_pool(name="xtp", bufs=2) as xtp,
            tc.tile_pool(name="rtmp", bufs=6) as rtmp,
            tc.tile_pool(name="ep", bufs=3) as ep,
            tc.tile_pool(name="rp", bufs=6) as rp,
            tc.tile_pool(name="osb", bufs=2) as osb,
            tc.tile_pool(name="pp", bufs=3, space="PSUM") as pp,
            tc.tile_pool(name="scp", bufs=3, space="PSUM") as scp,
            tc.tile_pool(name="avp", bufs=2, space="PSUM") as avp,
        ):
            # ---- constants ----
            wq_t = consts.tile([128, DCH, GD], F32R, tag="wq")
            wk_t = consts.tile([128, DCH, GD], F32R, tag="wk")
            wv_t = consts.tile([128, DCH, GD], F32R, tag="wv")
            wo_t = consts.tile([128, 2, D], F32R, tag="wo")
            cc_t = consts.tile([128, S], F32, tag="cc")
            ss_t = consts.tile([128, S], F32, tag="ss")
            nc.sync.dma_start(
                out=wq_t[:], in_=wq.rearrange("(c p) g -> p c g", p=128))
            nc.sync.dma_start(
                out=wk_t[:], in_=wk.rearrange("(c p) g -> p c g", p=128))
            nc.sync.dma_start(
                out=wv_t[:], in_=wv.rearrange("(c p) g -> p c g", p=128))
            nc.sync.dma_start(
                out=wo_t[:], in_=wo.rearrange("(c p) d -> p c d", p=128))
            nc.sync.dma_start(out=cc_t[:], in_=ccd[:])
            nc.sync.dma_start(out=ss_t[:], in_=ssd[:])

            ones_f = consts.tile([128, 2, DK], F32, tag="onesf")
            nc.vector.memset(ones_f[:], 1.0)
            ones = consts.tile([128, 2, DK], F32R, tag="ones")
            nc.vector.tensor_copy(ones[:], ones_f[:])

            # persistent activations
            qT = persist.tile([128, 2, S], F32R, tag="qT")
            kT = persist.tile([128, 2, S], F32R, tag="kT")
            yT = persist.tile([128, 2, S], F32R, tag="yT")
            v_ext = persist.tile([128, C, HL, 2 * DK], F32R, tag="vext")

            # ones halves of v_ext: even heads [64:128], odd heads [0:64]
            for c in range(C):
                for par, sl in ((0, slice(DK, 2 * DK)), (1, slice(0, DK))):
                    nc.sync.dma_start(
                        out=v_ext[:, c, par::2, sl], in_=ones[:])

            # ---- Phase A: projections + RoPE + v packing ----
            for t in range(T):
                qs = slice(512 * t, 512 * (t + 1))
                xt = xtp.tile([128, DCH, 512], F32R, tag="xt")
                nc.sync.dma_start(
                    out=xt[:],
                    in_=xT.rearrange("(c p) s -> p c s", p=128)[:, :, qs],
                )
                for w_t, dst in ((wq_t, qT), (wk_t, kT)):
                    pchunk = []
                    for oc in range(2):
                        ps = pp.tile([128, 512], F32, tag="pp")
                        for d in range(DCH):
                            nc.tensor.matmul(
                                ps[:],
                                lhsT=w_t[:, d, 128 * oc:128 * (oc + 1)],
                                rhs=xt[:, d, :],
                                start=(d == 0),
                                stop=(d == DCH - 1),
                            )
                        pchunk.append(ps)
                    p0, p1 = pchunk
                    ccs, sss = cc_t[:, qs], ss_t[:, qs]
                    t1 = rtmp.tile([128, 512], F32, tag="rt")
                    t2 = rtmp.tile([128, 512], F32, tag="rt")
                    t3 = rtmp.tile([128, 512], F32, tag="rt")
                    t4 = rtmp.tile([128, 512], F32, tag="rt")
                    nc.vector.tensor_tensor(t1[:], p0[:], ccs, op=MUL)
                    nc.vector.tensor_tensor(t2[:], p1[:], sss, op=MUL)
                    nc.vector.tensor_tensor(dst[:, 0, qs], t1[:], t2[:], op=SUB)
                    nc.vector.tensor_tensor(t3[:], p0[:], sss, op=MUL)
                    nc.vector.tensor_tensor(t4[:], p1[:], ccs, op=MUL)
                    nc.vector.tensor_tensor(dst[:, 1, qs], t3[:], t4[:], op=ADD)

                for s4 in range(4):
                    s = 4 * t + s4
                    psv = pp.tile([128, 512], F32, tag="pp")
                    for d in range(DCH):
                        nc.tensor.matmul(
                            psv[:, :GD],
                            lhsT=xt[:, d, 128 * s4:128 * (s4 + 1)],
                            rhs=wv_t[:, d, :],
                            start=(d == 0),
                            stop=(d == DCH - 1),
                        )
                    pv = psv[:, :GD].rearrange("p (h e) -> p h e", e=DK)
                    for par, sl in ((0, slice(0, DK)), (1, slice(DK, 2 * DK))):
                        nc.vector.tensor_copy(
                            v_ext[:, s, par::2, sl], pv[:, par::2, :])

            # ---- Phase B: attention ----
            for t in range(T):
                qs = slice(512 * t, 512 * (t + 1))
                for pair in range(2):
                    heads = (2 * pair, 2 * pair + 1)
                    av_ps = {}
                    for h in heads:
                        av_ps[h] = avp.tile([128, 512], F32, tag="av",
                                            name=f"av_{t}_{h}")
                    nck = 4 * t + 4  # eligible kpos chunks
                    for c in range(nck):
                        ks = slice(128 * c, 128 * (c + 1))
                        for h in heads:
                            hp = slice(32 * h, 32 * (h + 1))
                            sc = scp.tile([128, 512], F32, tag="sc")
                            nc.tensor.matmul(
                                sc[:],
                                lhsT=kT[hp, 0, ks],
                                rhs=qT[hp, 0, qs],
                                start=True, stop=False,
                                tile_position=(32 * h, 0),
                            )
                            nc.tensor.matmul(
                                sc[:],
                                lhsT=kT[hp, 1, ks],
                                rhs=qT[hp, 1, qs],
                                start=False, stop=True,
                                tile_position=(32 * h, 0),
                            )
                            e = ep.tile([128, 512], F32R, tag="e")
                            nc.scalar.activation(e[:], sc[:], EXP)
                            if c >= 4 * t:  # diagonal-crossing tile
                                nc.gpsimd.affine_select(
                                    out=e[:], in_=e[:],
                                    compare_op=mybir.AluOpType.is_ge,
                                    fill=0.0,
                                    base=512 * t - 128 * c,
                                    pattern=[[1, 512]],
                                    channel_multiplier=-1,
                                )
                            nc.tensor.matmul(
                                av_ps[h][:],
                                lhsT=v_ext[:, c, h, :],
                                rhs=e[:],
                                start=(c == 0),
                                stop=(c == nck - 1),
                            )
                    for h in heads:
                        # sums rows / out rows by head parity
                        if h % 2 == 0:
                            srows, orows = slice(64, 128), slice(0, 64)
                        else:
                            srows, orows = slice(0, 64), slice(64, 128)
                        r1 = rp.tile([128, 512], F32, tag="rr")
                        r2 = rp.tile([128, 512], F32, tag="rr")
                        r3 = rp.tile([128, 512], F32, tag="rr")
                        nc.scalar.activation(r1[srows], av_ps[h][srows], LN)
                        nc.scalar.activation(r2[srows], r1[srows], EXP,
                                             scale=-1.0)
                        nc.sync.dma_start(out=r3[orows], in_=r2[srows])
                        nc.vector.tensor_tensor(
                            yT[orows, h // 2, qs],
                            av_ps[h][orows], r3[orows], op=MUL)

            # ---- Phase C: output projection ----
            for s in range(C):
                ssl = slice(128 * s, 128 * (s + 1))
                for n in range(2):
                    nsl = slice(512 * n, 512 * (n + 1))
                    po = pp.tile([128, 512], F32, tag="pp")
                    for ldc in range(2):
                        nc.tensor.matmul(
                            po[:],
                            lhsT=yT[:, ldc, ssl],
                            rhs=wo_t[:, ldc, nsl],
                            start=(ldc == 0),
                            stop=(ldc == 1),
                        )
                    ob = osb.tile([128, 512], F32, tag="ob")
                    nc.vector.tensor_copy(ob[:], po[:])
                    nc.sync.dma_start(out=out[ssl, nsl], in_=ob[:])

    nc.compile()
    return nc


def _host_prep(x, token_positions, Wq, Wk, Wv, Wo):
    # d_k permutation folded into Wq/Wk: new row n (within a group slice of
    # 256) <- orig row 64*h + 2*j + chunk, with chunk = n//128, h = (n%128)//32,
    # j = n%32.  Chunk 0 = even (x1) lanes, chunk 1 = odd (x2) lanes.
    n = np.arange(GD)
    chunk = n // 128
    hh = (n % 128) // 32
    jj = n % 32
    perm = 64 * hh + 2 * jj + chunk

    pos = np.asarray(token_positions).astype(np.float64)
    inv_freq = THETA ** (-np.arange(0, DK, 2, dtype=np.float64) / DK)  # [32]
    ang = pos[:, None] * inv_freq[None, :]                             # [S, 32]
    cos = np.cos(ang).astype(np.float32)
    sin = np.sin(ang).astype(np.float32)
    # [128, S]: 4 replicated 32-row blocks, rows = freq j
    cc = np.ascontiguousarray(np.tile(cos.T, (4, 1)))
    ss = np.ascontiguousarray(np.tile(sin.T, (4, 1)))

    scale = 1.0 / np.sqrt(np.float32(DK))
    in_maps = []
    for core in range(8):
        b, g = divmod(core, 4)
        gsl = slice(GD * g, GD * (g + 1))
        in_maps.append({
            "xT": np.ascontiguousarray(np.asarray(x[b], np.float32).T),
            "wq": np.ascontiguousarray(
                (np.asarray(Wq[gsl], np.float32) * scale)[perm].T),
            "wk": np.ascontiguousarray(np.asarray(Wk[gsl], np.float32)[perm].T),
            "wv": np.ascontiguousarray(np.asarray(Wv[gsl], np.float32).T),
            "wo": np.ascontiguousarray(np.asarray(Wo[:, gsl], np.float32).T),
            "cc": cc,
            "ss": ss,
        })
    return in_maps


def kernel(x, token_positions, Wq, Wk, Wv, Wo, _trace=False, _result=[None],
           _tmpdir=None):
    if "nc" not in _cache:
        _cache["nc"] = _build_kernel()
    nc = _cache["nc"]
    in_maps = _host_prep(x, token_positions, Wq, Wk, Wv, Wo)
    res = run_bass_kernel_spmd(
        nc, in_maps, core_ids=list(range(8)), trace=_trace, tmpdir=_tmpdir)
    _result[0] = res
    outs = np.stack([r["out"] for r in res.results])  # [8, S, D]
    full = outs.reshape(B, 4, S, D).sum(axis=1, dtype=np.float32)
    return full
